# revision 67
# baseline (speedup 1.0000x reference)
"""Trainium2 Bass kernel for AdaptivePointMatcher (1024 pred x 512 gt point sets).

Sharding: data-parallel over the N=1024 pred rows across 8 NeuronCores (128
rows each); gt_points and the small MLP params are replicated. Softmax is
per-row over the full M=512, so no collectives are needed. The host does pure
layout/packing prep on the tiny inputs (transposes, block-diagonal layout,
bias replication, dtype conversion, fp8/fp16 scaling, packing everything into
two per-partition byte blobs so the device needs only 3 input DMAs); all
network compute runs on device.

Per-core device graph (all within one TileContext):
  1. Encoder: H^T = relu(W~^T X^T + b1) with W~ a (40,1280) block-diagonal
     packing of the per-point Linear(2,64), so PE cycles scale only with
     output columns; mean-over-P is folded into W2stack = [w2;w2]/20. Gives
     pred_f^T (128,128) and gt_f^T (128,512), feature-major bf16.
  2. a^T = (pred_f @ W1top)^T and B^T = (gt_f @ W1bot + b1)^T, both x64 so
     the fp8 cast of H1 stays in e4m3's normal range.
  3. Main loop over the 128 local pred rows i (grouped in threes):
       H1 = relu(B^T + a^T[:,i]) -> fp8e4; DVE tensor_scalar(add,max) hits
         the 2x port mode (~396ns/chunk); the first chunk is emitted as
         nc.any so Tile's dispatcher gap-fills whichever of DVE/ScalarE
         is idle at that moment (beats any static split)
       H2 = W2^T @ H1 in ONE fp8 DoubleRow matmul (K=256 virtualized)
       relu(H2) -> fp8 (ScalarE, one op per 3-i group, 3-bank PSUM tile)
       scores_i = H2r^T @ w3: 4 small fp8 matmuls, j-major, all 512 score
         columns accumulate into a single persistent PSUM bank
     DoubleRow matmuls are emitted under high_priority so the PE never
     blocks the relu chain behind stale score matmuls.
  4. Epilogue: expT = exp(1/16384 * scores) fp16 straight from PSUM (descale
     folded into the activation scale; no max-subtraction needed since
     |scores| < 0.01 by construction); matched and the softmax row-sums come
     from one matmul chain (gt_flat with a ones column appended, fp16);
     probs = transpose(expT) * (1/sums); confidence = rowmax(probs).

Accuracy vs the fp32 reference: ~3.8e-4 relative (fp8 is used only where the
softmax's tiny dynamic range leaves ~100x error margin).
"""

import numpy as np
from contextlib import ExitStack

N, M, P, D = 1024, 512, 20, 128
NCORES = 8
NLOC = N // NCORES  # 128
NCHUNK = (P * 64) // 128  # 10 feature chunks of the folded encoder hidden dim
BLOCK = 8  # pred rows per main-loop block

S1 = 64.0  # scale applied to H1 before fp8
SW2 = 16.0  # scale applied to W2 before fp8
SW3 = 16.0  # scale applied to w3
DESCALE = 1.0 / (S1 * SW2 * SW3)

# byte offsets inside the packed constant blobs (per partition, 64B-aligned)
# blob40: tensors living on partitions 0..39
OFF_XTG = 0       # xt_gt  bf16 (40, 512)   1024B
OFF_WT = 1024     # wt     bf16 (40, 1280)  2560B
BLOB40_B = 3584
# blob128: tensors on all 128 partitions
OFF_GTF = 0       # gtfo   fp16 (128, 4, 41) 328B -> pad 384
OFF_W2S = 384     # w2s    bf16 (128, 128)   256B
OFF_W1T = 640     # w1t    bf16 (128, 2, 128) 512B
OFF_W1B = 1152    # w1b    bf16 (128, 2, 128) 512B
OFF_W2PK = 1664   # w2pk   fp8  (128, 2, 128) 256B
OFF_IDENT = 1920  # ident  fp16 (128, 128)   256B
OFF_B1REP = 2176  # b1rep  f32  (128, 1)       4B
OFF_B2E = 2180    # b2e    f32  (128, 1)       4B
OFF_B1M = 2184    # b1m    f32  (128, 2)       8B
OFF_W3 = 2192     # w3pk   fp8 (128, 1)        1B
BLOB128_B = 2240

_CACHE: dict = {}


def _np_dt(dt_name):
    import concourse.mybir as mybir

    return mybir.dt.np(getattr(mybir.dt, dt_name))


def _prep_shared(inputs):
    """Host-side layout/packing of the replicated tensors (all tiny)."""
    bf = _np_dt("bfloat16")
    f8 = _np_dt("float8e4")
    f32 = np.float32
    gt = np.asarray(inputs["gt_points"], f32)  # (512, 20, 2)
    pe_w1 = np.asarray(inputs["pe_w1"], f32)
    pe_b1 = np.asarray(inputs["pe_b1"], f32)
    pe_w2 = np.asarray(inputs["pe_w2"], f32)
    pe_b2 = np.asarray(inputs["pe_b2"], f32)
    mn_w1 = np.asarray(inputs["mn_w1"], f32)
    mn_b1 = np.asarray(inputs["mn_b1"], f32)
    mn_w2 = np.asarray(inputs["mn_w2"], f32)
    mn_w3 = np.asarray(inputs["mn_w3"], f32)

    gt_flat = gt.reshape(M, P * 2)
    wt = np.zeros((40, P * 64), f32)
    for p in range(P):
        wt[2 * p : 2 * p + 2, 64 * p : 64 * p + 64] = pe_w1

    blob40 = np.zeros((40, BLOB40_B), np.uint8)
    blob128 = np.zeros((128, BLOB128_B), np.uint8)

    def place(blob, arr, off):
        a = np.ascontiguousarray(arr)
        b = a.view(np.uint8).reshape(a.shape[0], -1)
        blob[: b.shape[0], off : off + b.shape[1]] = b

    place(blob40, gt_flat.T.astype(bf), OFF_XTG)  # (40, 512)
    place(blob40, wt.astype(bf), OFF_WT)  # (40, 1280)
    f16 = np.float16
    gtfo = np.ones((128, 4, P * 2 + 1), f16)
    gtfo[:, :, : P * 2] = gt_flat.reshape(4, 128, P * 2).transpose(1, 0, 2).astype(f16)
    place(blob128, gtfo, OFF_GTF)
    place(blob128, (np.concatenate([pe_w2, pe_w2], 0) / P).astype(bf), OFF_W2S)
    place(blob128, (mn_w1[:128] * S1).reshape(128, 2, 128).astype(bf), OFF_W1T)
    place(blob128, (mn_w1[128:] * S1).reshape(128, 2, 128).astype(bf), OFF_W1B)
    place(blob128, (mn_w2 * SW2).reshape(2, 128, 128).transpose(1, 0, 2).astype(f8), OFF_W2PK)
    place(blob128, np.eye(128, dtype=f16), OFF_IDENT)
    place(blob128, np.tile(pe_b1, 2).reshape(128, 1).astype(f32), OFF_B1REP)
    place(blob128, pe_b2.reshape(128, 1).astype(f32), OFF_B2E)
    place(blob128, (mn_b1 * S1).reshape(2, 128).T.astype(f32), OFF_B1M)
    place(blob128, (mn_w3 * SW3).astype(f8), OFF_W3)
    return {"blob40": blob40, "blob128": blob128}


def _prep_pred(inputs, core):
    bf = _np_dt("bfloat16")
    pred = np.asarray(inputs["pred_points"], np.float32).reshape(N, P * 2)
    shard = pred[core * NLOC : (core + 1) * NLOC]  # (128, 40)
    return np.ascontiguousarray(shard.T.astype(bf))  # (40, 128)


def _build_nc():
    import concourse.bass as bass
    import concourse.mybir as mybir
    import concourse.tile as tile
    from concourse import bacc

    fp32 = mybir.dt.float32
    bf16 = mybir.dt.bfloat16
    fp16 = mybir.dt.float16
    fp8 = mybir.dt.float8e4
    AF = mybir.ActivationFunctionType
    OP = mybir.AluOpType

    nc = bacc.Bacc(
        "TRN2",
        target_bir_lowering=False,
        debug=False,
        enable_asserts=True,
        num_devices=NCORES,
    )

    # ---- DRAM I/O (host-prepped layouts) ----
    ui8 = mybir.dt.uint8
    d_xt_pred = nc.dram_tensor("xt_pred", (40, NLOC), bf16, kind="ExternalInput").ap()
    d_blob40 = nc.dram_tensor("blob40", (40, BLOB40_B), ui8, kind="ExternalInput").ap()
    d_blob128 = nc.dram_tensor("blob128", (128, BLOB128_B), ui8, kind="ExternalInput").ap()

    out_matched = nc.dram_tensor("matched", (NLOC, P, 2), fp32, kind="ExternalOutput").ap()
    out_conf = nc.dram_tensor("confidence", (NLOC, 1), fp32, kind="ExternalOutput").ap()
    out_probs = nc.dram_tensor("probs", (NLOC, M), fp32, kind="ExternalOutput").ap()

    with tile.TileContext(nc) as tc, ExitStack() as ctx:
        const = ctx.enter_context(tc.tile_pool(name="const", bufs=1))

        # ---------- persistent tiles + input DMAs ----------
        xt_pred = const.tile([40, NLOC], bf16)
        blob40 = const.tile([40, BLOB40_B], ui8)
        blob128 = const.tile([128, BLOB128_B], ui8)

        hg_sb = const.tile([128, NCHUNK, M], bf16)
        hp_sb = const.tile([128, NCHUNK, NLOC], bf16)
        predf_sb = const.tile([128, NLOC], bf16)
        gtf_feat = const.tile([128, M], bf16)
        at_sb = const.tile([128, 2, NLOC], fp32)
        bt_sb = const.tile([128, 2, M], fp32)

        nc.sync.dma_start(blob40[:], d_blob40[:, :])
        nc.sync.dma_start(xt_pred[:], d_xt_pred[:, :])
        nc.sync.dma_start(blob128[:], d_blob128[:, :])

        def bslice(off, nbytes, dt, rows=128):
            return blob128[0:rows, off : off + nbytes].bitcast(dt)

        xt_gt = blob40[0:40, OFF_XTG : OFF_XTG + 1024].bitcast(bf16)  # (40, 512)
        wt_bf = blob40[0:40, OFF_WT : OFF_WT + 2560].bitcast(bf16)  # (40, 1280)
        gtfo = bslice(OFF_GTF, 328, fp16).rearrange("p (c q) -> p c q", c=4)
        w2s_bf = bslice(OFF_W2S, 256, bf16)
        w1t_bf = bslice(OFF_W1T, 512, bf16).rearrange("p (c k) -> p c k", c=2)
        w1b_bf = bslice(OFF_W1B, 512, bf16).rearrange("p (c k) -> p c k", c=2)
        w2pk = bslice(OFF_W2PK, 256, fp8).rearrange("p (c k) -> p c k", c=2)
        id_f16 = bslice(OFF_IDENT, 256, fp16)
        b1rep = bslice(OFF_B1REP, 4, fp32)
        b2e = bslice(OFF_B2E, 4, fp32)
        b1s = bslice(OFF_B1M, 8, fp32)
        w3pk = bslice(OFF_W3, 1, fp8)

        # warm the ACT exp table set early (one-time ~2.7us load overlaps encoder)
        scratch = const.tile([128, 1], fp32)
        nc.vector.memset(scratch[:], 0.0)
        nc.scalar.activation(scratch[:], scratch[:], AF.Exp)

        # ---------- encoder ----------
        with tc.tile_pool(name="encpsum", bufs=2, space="PSUM") as encpsum, \
             tc.tile_pool(name="encacc", bufs=1, space="PSUM") as encacc:
            for c2 in range(NCHUNK // 2):
                hps = encpsum.tile([128, 2, M], fp32, tag="hps")
                hpp = encpsum.tile([128, 2, NLOC], fp32, tag="hpp")
                for r in range(2):
                    lhs = wt_bf[:, 128 * (2 * c2 + r) : 128 * (2 * c2 + r + 1)]
                    nc.tensor.matmul(hps[:, r, :], lhsT=lhs, rhs=xt_gt[:], start=True, stop=True)
                    nc.tensor.matmul(hpp[:, r, :], lhsT=lhs, rhs=xt_pred[:], start=True, stop=True)
                if c2 % 2 == 0:
                    nc.scalar.activation(hg_sb[:, 2 * c2 : 2 * c2 + 2, :], hps[:], AF.Relu, bias=b1rep[:])
                    nc.vector.tensor_scalar(
                        hp_sb[:, 2 * c2 : 2 * c2 + 2, :], hpp[:], b1rep[:, 0:1], 0.0,
                        op0=OP.add, op1=OP.max,
                    )
                else:
                    nc.vector.tensor_scalar(
                        hg_sb[:, 2 * c2 : 2 * c2 + 2, :], hps[:], b1rep[:, 0:1], 0.0,
                        op0=OP.add, op1=OP.max,
                    )
                    nc.scalar.activation(hp_sb[:, 2 * c2 : 2 * c2 + 2, :], hpp[:], AF.Relu, bias=b1rep[:])

            # layer 2 (+ mean fold)
            pfps = encacc.tile([128, NLOC], fp32, tag="accp")
            for c in range(NCHUNK):
                nc.tensor.matmul(
                    pfps[:], lhsT=w2s_bf[:], rhs=hp_sb[:, c, :],
                    start=(c == 0), stop=(c == NCHUNK - 1),
                )
            nc.scalar.activation(predf_sb[:], pfps[:], AF.Identity, bias=b2e[:])
            gfps = encacc.tile([128, M], fp32, tag="accg")
            for c in range(NCHUNK):
                nc.tensor.matmul(
                    gfps[:], lhsT=w2s_bf[:], rhs=hg_sb[:, c, :],
                    start=(c == 0), stop=(c == NCHUNK - 1),
                )
            nc.scalar.activation(gtf_feat[:], gfps[:], AF.Identity, bias=b2e[:])

            # a^T and B^T (scaled x64 via host-scaled w1t/w1b)
            for c in range(2):
                atps = encacc.tile([128, NLOC], fp32, tag="accp")
                nc.tensor.matmul(atps[:], lhsT=w1t_bf[:, c, :], rhs=predf_sb[:], start=True, stop=True)
                nc.vector.tensor_copy(at_sb[:, c, :], atps[:])
                btps = encacc.tile([128, M], fp32, tag="accg")
                nc.tensor.matmul(btps[:], lhsT=w1b_bf[:, c, :], rhs=gtf_feat[:], start=True, stop=True)
                nc.scalar.activation(bt_sb[:, c, :], btps[:], AF.Identity, bias=b1s[:, c : c + 1])

        # ---------- main loop ----------
        h1_pool = ctx.enter_context(tc.tile_pool(name="h1", bufs=14))
        h2sb_pool = ctx.enter_context(tc.tile_pool(name="h2sb", bufs=8))
        mainps = ExitStack()
        h2ps_pool = mainps.enter_context(tc.tile_pool(name="h2ps", bufs=2, space="PSUM"))
        scorps_pool = mainps.enter_context(tc.tile_pool(name="scorps", bufs=1, space="PSUM"))
        DR = mybir.MatmulPerfMode.DoubleRow

        # all 512 per-i L3 score columns land in one persistent PSUM bank,
        # laid out (jp, i, jc) so the epilogue can read it j-chunk-major
        scores_ps = scorps_pool.tile([128, NLOC, 4], fp32)
        GROUPS = [3] * 40 + [2] * 4  # i's per relu group (3-bank psum tiles)

        def emit_l3(h2sb, i0, glen):
            for r in range(glen):
                for c in range(4):
                    nc.tensor.matmul(
                        scores_ps[:, i0 + r, c : c + 1],
                        lhsT=h2sb[:, r, 128 * c : 128 * (c + 1)],
                        rhs=w3pk[:], start=True, stop=True,
                    )

        prev_h2 = None
        i0 = 0
        for glen in GROUPS:
            h2ps = h2ps_pool.tile([128, glen, M], fp32, tag="h2ps")
            h2sb = h2sb_pool.tile([128, glen, M], fp8, tag="h2sb")
            for r in range(glen):
                i = i0 + r
                h1 = h1_pool.tile([128, 2, M], fp8)
                nc.any.tensor_scalar(
                    h1[:, 0, :], bt_sb[:, 0, :], at_sb[:, 0, i : i + 1], 0.0,
                    op0=OP.add, op1=OP.max,
                )
                nc.vector.tensor_scalar(
                    h1[:, 1, :], bt_sb[:, 1, :], at_sb[:, 1, i : i + 1], 0.0,
                    op0=OP.add, op1=OP.max,
                )
                with tc.high_priority(offset=150):
                    nc.tensor.matmul(
                        h2ps[:, r, :], lhsT=w2pk[:], rhs=h1[:], perf_mode=DR, start=True, stop=True
                    )
            if prev_h2 is not None:
                emit_l3(*prev_h2)
            nc.scalar.activation(h2sb[:], h2ps[:], AF.Relu)
            prev_h2 = (h2sb, i0, glen)
            i0 += glen
        emit_l3(*prev_h2)

        # ---------- softmax epilogue ----------
        # expT = exp(descale * scores) straight from PSUM, fp16, j-major
        epi = ctx.enter_context(tc.tile_pool(name="epi", bufs=1))
        expt = epi.tile([128, 4, NLOC], fp16)
        nc.scalar.activation(
            expt[:], scores_ps[:].rearrange("p i c -> p c i"), AF.Exp, scale=DESCALE
        )
        mainps.close()
        with tc.tile_pool(name="episum", bufs=1, space="PSUM") as episum:
            # matched+sums in one chain: rhs has a ones column appended
            mps = episum.tile([128, P * 2 + 1], fp32, tag="mps")
            for c in range(4):
                nc.tensor.matmul(
                    mps[:], lhsT=expt[:, c, :], rhs=gtfo[:, c, :],
                    start=(c == 0), stop=(c == 3),
                )
            rs = epi.tile([128, 1], fp32)
            nc.vector.reciprocal(rs[:], mps[:, P * 2 : P * 2 + 1])
            matched_sb = epi.tile([128, P * 2], fp32)
            nc.vector.tensor_scalar(matched_sb[:], mps[:, : P * 2], rs[:], None, op0=OP.mult)
            nc.sync.dma_start(out_matched.rearrange("n p t -> n (p t)"), matched_sb[:])

            # probs: transpose exp to i-major, scale by 1/sum
            exp_im = episum.tile([128, M], fp16, tag="expim")
            for c in range(4):
                nc.tensor.transpose(
                    exp_im[:, 128 * c : 128 * (c + 1)], expt[:, c, :], id_f16[:]
                )
            probs_sb = epi.tile([128, M], fp32)
            nc.vector.tensor_scalar(probs_sb[:], exp_im[:], rs[:], None, op0=OP.mult)
            nc.sync.dma_start(out_probs[:, :], probs_sb[:])
            conf_sb = epi.tile([128, 1], fp32)
            nc.vector.reduce_max(conf_sb[:], probs_sb[:], axis=mybir.AxisListType.X)
            nc.sync.dma_start(out_conf[:, :], conf_sb[:])

    nc.compile()
    return nc


def _get_nc():
    if "nc" not in _CACHE:
        _CACHE["nc"] = _build_nc()
    return _CACHE["nc"]


def make_in_maps(inputs):
    shared = _prep_shared(inputs)
    in_maps = []
    for c in range(NCORES):
        m = dict(shared)
        m["xt_pred"] = _prep_pred(inputs, c)
        in_maps.append(m)
    return in_maps


def kernel(**inputs) -> tuple:
    nc = _get_nc()
    from concourse import bass_utils

    in_maps = make_in_maps(inputs)
    res = bass_utils.run_bass_kernel_spmd(nc, in_maps, core_ids=list(range(NCORES)))
    matched = np.concatenate([r["matched"] for r in res.results], axis=0)
    confidence = np.concatenate([r["confidence"] for r in res.results], axis=0)
    probs = np.concatenate([r["probs"] for r in res.results], axis=0)
    return matched, confidence, probs


# revision 68
# speedup vs baseline: 1.0284x; 1.0284x over previous
"""Trainium2 Bass kernel for AdaptivePointMatcher (1024 pred x 512 gt point sets).

Sharding: data-parallel over the N=1024 pred rows across 8 NeuronCores (128
rows each); gt_points and the small MLP params are replicated. Softmax is
per-row over the full M=512, so no collectives are needed. The host does pure
layout/packing prep on the tiny inputs (transposes, block-diagonal layout,
bias replication, dtype conversion, fp8/fp16 scaling, packing everything into
two per-partition byte blobs so the device needs only 3 input DMAs); all
network compute runs on device.

Per-core device graph (all within one TileContext):
  1. Encoder: H^T = relu(W~^T X^T + b1) with W~ a (40,1280) block-diagonal
     packing of the per-point Linear(2,64), so PE cycles scale only with
     output columns; mean-over-P is folded into W2stack = [w2;w2]/20. Gives
     pred_f^T (128,128) and gt_f^T (128,512), feature-major bf16.
  2. a^T = (pred_f @ W1top)^T and B^T = (gt_f @ W1bot + b1)^T, both x64 so
     the fp8 cast of H1 stays in e4m3's normal range.
  3. Main loop over the 128 local pred rows i (grouped in threes):
       H1 = relu(B^T + a^T[:,i]) -> fp8e4; DVE tensor_scalar(add,max) hits
         the 2x port mode (~396ns/chunk); the first chunk is emitted as
         nc.any so Tile's dispatcher gap-fills whichever of DVE/ScalarE
         is idle at that moment (beats any static split)
       H2 = W2^T @ H1 in ONE fp8 DoubleRow matmul (K=256 virtualized)
       relu(H2) -> fp8 (ScalarE, one op per 3-i group, 3-bank PSUM tile)
       scores_i = H2r^T @ w3: 4 small fp8 matmuls, j-major, all 512 score
         columns accumulate into a single persistent PSUM bank
     DoubleRow matmuls are emitted under high_priority so the PE never
     blocks the relu chain behind stale score matmuls.
  4. Epilogue: expT = exp(1/16384 * scores) fp16 straight from PSUM (descale
     folded into the activation scale; no max-subtraction needed since
     |scores| < 0.01 by construction); matched and the softmax row-sums come
     from one matmul chain (gt_flat with a ones column appended, fp16);
     probs = transpose(expT) * (1/sums); confidence = rowmax(probs).

Accuracy vs the fp32 reference: ~3.8e-4 relative (fp8 is used only where the
softmax's tiny dynamic range leaves ~100x error margin).
"""

import numpy as np
from contextlib import ExitStack

N, M, P, D = 1024, 512, 20, 128
NCORES = 8
NLOC = N // NCORES  # 128
NCHUNK = (P * 64) // 128  # 10 feature chunks of the folded encoder hidden dim
BLOCK = 8  # pred rows per main-loop block

S1 = 64.0  # scale applied to H1 before fp8
SW2 = 16.0  # scale applied to W2 before fp8
SW3 = 16.0  # scale applied to w3
DESCALE = 1.0 / (S1 * SW2 * SW3)

# byte offsets inside the packed constant blobs (per partition, 64B-aligned)
# blob40: tensors living on partitions 0..39
OFF_XTG = 0       # xt_gt  bf16 (40, 512)   1024B
OFF_WT = 1024     # wt     bf16 (40, 1280)  2560B
BLOB40_B = 3584
# blob128: tensors on all 128 partitions
OFF_GTF = 0       # gtfo   fp16 (128, 4, 41) 328B -> pad 384
OFF_W2S = 384     # w2s    bf16 (128, 128)   256B
OFF_W1T = 640     # w1t    bf16 (128, 2, 128) 512B
OFF_W1B = 1152    # w1b    bf16 (128, 2, 128) 512B
OFF_W2PK = 1664   # w2pk   fp8  (128, 2, 128) 256B
OFF_IDENT = 1920  # ident  fp16 (128, 128)   256B
OFF_B1REP = 2176  # b1rep  f32  (128, 1)       4B
OFF_B2E = 2180    # b2e    f32  (128, 1)       4B
OFF_B1M = 2184    # b1m    f32  (128, 2)       8B
OFF_W3 = 2192     # w3pk   fp8 (128, 1)        1B
BLOB128_B = 2240

_CACHE: dict = {}


def _np_dt(dt_name):
    import concourse.mybir as mybir

    return mybir.dt.np(getattr(mybir.dt, dt_name))


def _prep_shared(inputs):
    """Host-side layout/packing of the replicated tensors (all tiny)."""
    bf = _np_dt("bfloat16")
    f8 = _np_dt("float8e4")
    f32 = np.float32
    gt = np.asarray(inputs["gt_points"], f32)  # (512, 20, 2)
    pe_w1 = np.asarray(inputs["pe_w1"], f32)
    pe_b1 = np.asarray(inputs["pe_b1"], f32)
    pe_w2 = np.asarray(inputs["pe_w2"], f32)
    pe_b2 = np.asarray(inputs["pe_b2"], f32)
    mn_w1 = np.asarray(inputs["mn_w1"], f32)
    mn_b1 = np.asarray(inputs["mn_b1"], f32)
    mn_w2 = np.asarray(inputs["mn_w2"], f32)
    mn_w3 = np.asarray(inputs["mn_w3"], f32)

    gt_flat = gt.reshape(M, P * 2)
    wt = np.zeros((40, P * 64), f32)
    for p in range(P):
        wt[2 * p : 2 * p + 2, 64 * p : 64 * p + 64] = pe_w1

    blob40 = np.zeros((40, BLOB40_B), np.uint8)
    blob128 = np.zeros((128, BLOB128_B), np.uint8)

    def place(blob, arr, off):
        a = np.ascontiguousarray(arr)
        b = a.view(np.uint8).reshape(a.shape[0], -1)
        blob[: b.shape[0], off : off + b.shape[1]] = b

    place(blob40, gt_flat.T.astype(bf), OFF_XTG)  # (40, 512)
    place(blob40, wt.astype(bf), OFF_WT)  # (40, 1280)
    f16 = np.float16
    gtfo = np.ones((128, 4, P * 2 + 1), f16)
    gtfo[:, :, : P * 2] = gt_flat.reshape(4, 128, P * 2).transpose(1, 0, 2).astype(f16)
    place(blob128, gtfo, OFF_GTF)
    place(blob128, (np.concatenate([pe_w2, pe_w2], 0) / P).astype(bf), OFF_W2S)
    place(blob128, (mn_w1[:128] * S1).reshape(128, 2, 128).astype(bf), OFF_W1T)
    place(blob128, (mn_w1[128:] * S1).reshape(128, 2, 128).astype(bf), OFF_W1B)
    place(blob128, (mn_w2 * SW2).reshape(2, 128, 128).transpose(1, 0, 2).astype(f8), OFF_W2PK)
    place(blob128, np.eye(128, dtype=f16), OFF_IDENT)
    place(blob128, np.tile(pe_b1, 2).reshape(128, 1).astype(f32), OFF_B1REP)
    place(blob128, pe_b2.reshape(128, 1).astype(f32), OFF_B2E)
    place(blob128, (mn_b1 * S1).reshape(2, 128).T.astype(f32), OFF_B1M)
    place(blob128, (mn_w3 * SW3).astype(f8), OFF_W3)
    return {"blob40": blob40, "blob128": blob128}


def _prep_pred(inputs, core):
    bf = _np_dt("bfloat16")
    pred = np.asarray(inputs["pred_points"], np.float32).reshape(N, P * 2)
    shard = pred[core * NLOC : (core + 1) * NLOC]  # (128, 40)
    return np.ascontiguousarray(shard.T.astype(bf))  # (40, 128)


def _build_nc():
    import concourse.bass as bass
    import concourse.mybir as mybir
    import concourse.tile as tile
    from concourse import bacc

    fp32 = mybir.dt.float32
    bf16 = mybir.dt.bfloat16
    fp16 = mybir.dt.float16
    fp8 = mybir.dt.float8e4
    AF = mybir.ActivationFunctionType
    OP = mybir.AluOpType

    nc = bacc.Bacc(
        "TRN2",
        target_bir_lowering=False,
        debug=False,
        enable_asserts=True,
        num_devices=NCORES,
    )

    # ---- DRAM I/O (host-prepped layouts) ----
    ui8 = mybir.dt.uint8
    d_xt_pred = nc.dram_tensor("xt_pred", (40, NLOC), bf16, kind="ExternalInput").ap()
    d_blob40 = nc.dram_tensor("blob40", (40, BLOB40_B), ui8, kind="ExternalInput").ap()
    d_blob128 = nc.dram_tensor("blob128", (128, BLOB128_B), ui8, kind="ExternalInput").ap()

    out_matched = nc.dram_tensor("matched", (NLOC, P, 2), fp32, kind="ExternalOutput").ap()
    out_conf = nc.dram_tensor("confidence", (NLOC, 1), fp32, kind="ExternalOutput").ap()
    out_probs = nc.dram_tensor("probs", (NLOC, M), fp32, kind="ExternalOutput").ap()

    with tile.TileContext(nc) as tc, ExitStack() as ctx:
        const = ctx.enter_context(tc.tile_pool(name="const", bufs=1))

        # ---------- persistent tiles + input DMAs ----------
        xt_pred = const.tile([40, NLOC], bf16)
        blob40 = const.tile([40, BLOB40_B], ui8)
        blob128 = const.tile([128, BLOB128_B], ui8)

        hg_sb = const.tile([128, NCHUNK, M], bf16)
        hp_sb = const.tile([128, NCHUNK, NLOC], bf16)
        predf_sb = const.tile([128, NLOC], bf16)
        gtf_feat = const.tile([128, M], bf16)
        at_sb = const.tile([128, 2, NLOC], fp32)
        bt_sb = const.tile([128, 2, M], fp32)

        nc.sync.dma_start(blob40[:], d_blob40[:, :])
        nc.sync.dma_start(xt_pred[:], d_xt_pred[:, :])
        nc.sync.dma_start(blob128[:], d_blob128[:, :])

        def bslice(off, nbytes, dt, rows=128):
            return blob128[0:rows, off : off + nbytes].bitcast(dt)

        xt_gt = blob40[0:40, OFF_XTG : OFF_XTG + 1024].bitcast(bf16)  # (40, 512)
        wt_bf = blob40[0:40, OFF_WT : OFF_WT + 2560].bitcast(bf16)  # (40, 1280)
        gtfo = bslice(OFF_GTF, 328, fp16).rearrange("p (c q) -> p c q", c=4)
        w2s_bf = bslice(OFF_W2S, 256, bf16)
        w1t_bf = bslice(OFF_W1T, 512, bf16).rearrange("p (c k) -> p c k", c=2)
        w1b_bf = bslice(OFF_W1B, 512, bf16).rearrange("p (c k) -> p c k", c=2)
        w2pk = bslice(OFF_W2PK, 256, fp8).rearrange("p (c k) -> p c k", c=2)
        id_f16 = bslice(OFF_IDENT, 256, fp16)
        b1rep = bslice(OFF_B1REP, 4, fp32)
        b2e = bslice(OFF_B2E, 4, fp32)
        b1s = bslice(OFF_B1M, 8, fp32)
        w3pk = bslice(OFF_W3, 1, fp8)

        # warm the ACT exp table set early (one-time ~2.7us load overlaps encoder)
        scratch = const.tile([128, 1], fp32)
        nc.vector.memset(scratch[:], 0.0)
        nc.scalar.activation(scratch[:], scratch[:], AF.Exp)

        # ---------- encoder ----------
        with tc.tile_pool(name="encpsum", bufs=2, space="PSUM") as encpsum, \
             tc.tile_pool(name="encacc", bufs=1, space="PSUM") as encacc:
            for c2 in range(NCHUNK // 2):
                hps = encpsum.tile([128, 2, M], fp32, tag="hps")
                hpp = encpsum.tile([128, 2, NLOC], fp32, tag="hpp")
                for r in range(2):
                    lhs = wt_bf[:, 128 * (2 * c2 + r) : 128 * (2 * c2 + r + 1)]
                    nc.tensor.matmul(hps[:, r, :], lhsT=lhs, rhs=xt_gt[:], start=True, stop=True)
                    nc.tensor.matmul(hpp[:, r, :], lhsT=lhs, rhs=xt_pred[:], start=True, stop=True)
                if c2 % 2 == 0:
                    nc.scalar.activation(hg_sb[:, 2 * c2 : 2 * c2 + 2, :], hps[:], AF.Relu, bias=b1rep[:])
                    nc.vector.tensor_scalar(
                        hp_sb[:, 2 * c2 : 2 * c2 + 2, :], hpp[:], b1rep[:, 0:1], 0.0,
                        op0=OP.add, op1=OP.max,
                    )
                else:
                    nc.vector.tensor_scalar(
                        hg_sb[:, 2 * c2 : 2 * c2 + 2, :], hps[:], b1rep[:, 0:1], 0.0,
                        op0=OP.add, op1=OP.max,
                    )
                    nc.scalar.activation(hp_sb[:, 2 * c2 : 2 * c2 + 2, :], hpp[:], AF.Relu, bias=b1rep[:])

            # layer 2 (+ mean fold)
            pfps = encacc.tile([128, NLOC], fp32, tag="accp")
            for c in range(NCHUNK):
                nc.tensor.matmul(
                    pfps[:], lhsT=w2s_bf[:], rhs=hp_sb[:, c, :],
                    start=(c == 0), stop=(c == NCHUNK - 1),
                )
            nc.scalar.activation(predf_sb[:], pfps[:], AF.Identity, bias=b2e[:])
            gfps = encacc.tile([128, M], fp32, tag="accg")
            for c in range(NCHUNK):
                nc.tensor.matmul(
                    gfps[:], lhsT=w2s_bf[:], rhs=hg_sb[:, c, :],
                    start=(c == 0), stop=(c == NCHUNK - 1),
                )
            nc.scalar.activation(gtf_feat[:], gfps[:], AF.Identity, bias=b2e[:])

            # a^T and B^T (scaled x64 via host-scaled w1t/w1b)
            for c in range(2):
                atps = encacc.tile([128, NLOC], fp32, tag="accp")
                nc.tensor.matmul(atps[:], lhsT=w1t_bf[:, c, :], rhs=predf_sb[:], start=True, stop=True)
                nc.vector.tensor_copy(at_sb[:, c, :], atps[:])
                btps = encacc.tile([128, M], fp32, tag="accg")
                nc.tensor.matmul(btps[:], lhsT=w1b_bf[:, c, :], rhs=gtf_feat[:], start=True, stop=True)
                nc.scalar.activation(bt_sb[:, c, :], btps[:], AF.Identity, bias=b1s[:, c : c + 1])

        # ---------- main loop ----------
        h1_pool = ctx.enter_context(tc.tile_pool(name="h1", bufs=16))
        h2sb_pool = ctx.enter_context(tc.tile_pool(name="h2sb", bufs=8))
        mainps = ExitStack()
        h2ps_pool = mainps.enter_context(tc.tile_pool(name="h2ps", bufs=2, space="PSUM"))
        scorps_pool = mainps.enter_context(tc.tile_pool(name="scorps", bufs=1, space="PSUM"))
        DR = mybir.MatmulPerfMode.DoubleRow

        # all 512 per-i L3 score columns land in one persistent PSUM bank,
        # laid out (jp, i, jc) so the epilogue can read it j-chunk-major
        scores_ps = scorps_pool.tile([128, NLOC, 4], fp32)
        GROUPS = [3] * 40 + [2] * 4  # i's per relu group (3-bank psum tiles)

        def emit_l3(h2sb, i0, glen):
            for r in range(glen):
                for c in range(4):
                    nc.tensor.matmul(
                        scores_ps[:, i0 + r, c : c + 1],
                        lhsT=h2sb[:, r, 128 * c : 128 * (c + 1)],
                        rhs=w3pk[:], start=True, stop=True,
                    )

        prev_h2 = None
        i0 = 0
        for glen in GROUPS:
            h2ps = h2ps_pool.tile([128, glen, M], fp32, tag="h2ps")
            h2sb = h2sb_pool.tile([128, glen, M], fp8, tag="h2sb")
            for r in range(glen):
                i = i0 + r
                h1 = h1_pool.tile([128, 2, M], fp8)
                with tc.high_priority(offset=150):
                    nc.any.tensor_scalar(
                        h1[:, 0, :], bt_sb[:, 0, :], at_sb[:, 0, i : i + 1], 0.0,
                        op0=OP.add, op1=OP.max,
                    )
                    nc.vector.tensor_scalar(
                        h1[:, 1, :], bt_sb[:, 1, :], at_sb[:, 1, i : i + 1], 0.0,
                        op0=OP.add, op1=OP.max,
                    )
                with tc.high_priority(offset=150):
                    nc.tensor.matmul(
                        h2ps[:, r, :], lhsT=w2pk[:], rhs=h1[:], perf_mode=DR, start=True, stop=True
                    )
            if prev_h2 is not None:
                emit_l3(*prev_h2)
            nc.scalar.activation(h2sb[:], h2ps[:], AF.Relu)
            prev_h2 = (h2sb, i0, glen)
            i0 += glen
        emit_l3(*prev_h2)

        # ---------- softmax epilogue ----------
        # expT = exp(descale * scores) straight from PSUM, fp16, j-major
        epi = ctx.enter_context(tc.tile_pool(name="epi", bufs=1))
        expt = epi.tile([128, 4, NLOC], fp16)
        nc.scalar.activation(
            expt[:], scores_ps[:].rearrange("p i c -> p c i"), AF.Exp, scale=DESCALE
        )
        mainps.close()
        with tc.tile_pool(name="episum", bufs=1, space="PSUM") as episum:
            # matched+sums in one chain: rhs has a ones column appended
            mps = episum.tile([128, P * 2 + 1], fp32, tag="mps")
            for c in range(4):
                nc.tensor.matmul(
                    mps[:], lhsT=expt[:, c, :], rhs=gtfo[:, c, :],
                    start=(c == 0), stop=(c == 3),
                )
            rs = epi.tile([128, 1], fp32)
            nc.vector.reciprocal(rs[:], mps[:, P * 2 : P * 2 + 1])
            matched_sb = epi.tile([128, P * 2], fp32)
            nc.vector.tensor_scalar(matched_sb[:], mps[:, : P * 2], rs[:], None, op0=OP.mult)
            nc.sync.dma_start(out_matched.rearrange("n p t -> n (p t)"), matched_sb[:])

            # probs: transpose exp to i-major, scale by 1/sum
            exp_im = episum.tile([128, M], fp16, tag="expim")
            for c in range(4):
                nc.tensor.transpose(
                    exp_im[:, 128 * c : 128 * (c + 1)], expt[:, c, :], id_f16[:]
                )
            probs_sb = epi.tile([128, M], fp32)
            nc.vector.tensor_scalar(probs_sb[:], exp_im[:], rs[:], None, op0=OP.mult)
            nc.sync.dma_start(out_probs[:, :], probs_sb[:])
            conf_sb = epi.tile([128, 1], fp32)
            nc.vector.reduce_max(conf_sb[:], probs_sb[:], axis=mybir.AxisListType.X)
            nc.sync.dma_start(out_conf[:, :], conf_sb[:])

    nc.compile()
    return nc


def _get_nc():
    if "nc" not in _CACHE:
        _CACHE["nc"] = _build_nc()
    return _CACHE["nc"]


def make_in_maps(inputs):
    shared = _prep_shared(inputs)
    in_maps = []
    for c in range(NCORES):
        m = dict(shared)
        m["xt_pred"] = _prep_pred(inputs, c)
        in_maps.append(m)
    return in_maps


def kernel(**inputs) -> tuple:
    nc = _get_nc()
    from concourse import bass_utils

    in_maps = make_in_maps(inputs)
    res = bass_utils.run_bass_kernel_spmd(nc, in_maps, core_ids=list(range(NCORES)))
    matched = np.concatenate([r["matched"] for r in res.results], axis=0)
    confidence = np.concatenate([r["confidence"] for r in res.results], axis=0)
    probs = np.concatenate([r["probs"] for r in res.results], axis=0)
    return matched, confidence, probs
